# revision 22
# baseline (speedup 1.0000x reference)
"""Trainium2 (Bass/Tile) segment-sum kernel, 8-core SPMD.

Computes out[v, :] = sum over rows n with X_node[n] == v of H[n, :]
(equivalent to jax.ops.segment_sum(H, X_node, num_segments=V)).

Strategy:
  host: stable-argsort rows by segment id; split the sorted order into 8
    contiguous chunks (one per NeuronCore) so each core covers a narrow,
    contiguous segment range (~V/8 segments). Within a core, rows are
    greedily grouped into W windows, each covering <=128 consecutive
    segments and <=T*128 rows; each window is laid out as T tiles of 128
    rows, padded with dummy rows (lid=255) so all 8 cores run ONE static
    SPMD program. The DRAM layout is pre-swizzled so each window is a
    single descriptor-efficient DMA ([128 partitions] x contiguous runs).
  precision: H is split as H = hi + lo with hi = bf16(H) and
    lo = bf16(H - hi) (two bf16 planes = same DMA bytes as f32, ~2^-17
    relative representation error). The one-hot weights are exactly 0/1
    in bf16, and the PE accumulates in fp32 PSUM, so the result matches
    the f32 reference to ~1e-5 relative. bf16 matmuls run the PE at
    2.4 GHz with hidden weight loads (~59 ns per 128x128x128), vs fp32's
    ~224 ns -- this moves the kernel from PE-bound to DMA-bound.
  device, per (window w, tile t): VectorE builds the one-hot stationary
    matrix onehot[n, v] = (lid[n] == v) with one fused is_equal per
    window; TensorE accumulates PSUM[v, d] += onehot^T @ Hhi_tile
    + onehot^T @ Hlo_tile over the window's T tiles (a cross-partition
    segmented reduce); PSUM is copied to SBUF and DMA'd out per window.
  host: add the per-core [W, 128, D] window strips into the full [V, D]
    output (windows of adjacent cores may overlap; addition is exact).

Measured on the target data: ~310-330 us HW exec across 8 cores (f32 DMA
roofline ~290 us), relative error 2.5e-6 vs the f32 reference. Setting
SEGSUM_PLANES=1 ships H as a single bf16 plane instead of hi+lo
(~233 us, relative error ~1.7e-3).
"""

import os

import numpy as np
from contextlib import ExitStack

import ml_dtypes
import concourse.bass as bass
import concourse.tile as tile
from concourse import bacc, mybir
from concourse.bass_utils import run_bass_kernel_spmd

F32 = mybir.dt.float32
BF16 = mybir.dt.bfloat16
NP_BF16 = ml_dtypes.bfloat16
P = 128  # partitions / tile rows / max window width (segments)
D = 128  # feature dim
N_CORES = 8
T_CANDIDATES = (28, 29, 30, 31, 32)  # tiles (of 128 rows) per window
PAD_LID = 255.0

LAST_RESULTS = None  # test-harness hook: BassKernelResults of the last run
_NC_CACHE = {}  # (W, T, planes) -> compiled Bacc program


def _build_nc_cached(W: int, T: int, planes: int):
    key = (W, T, planes)
    if key not in _NC_CACHE:
        _NC_CACHE[key] = _build_nc(W, T, planes)
    return _NC_CACHE[key]


def _build_nc(W: int, T: int, planes: int):
    nc = bacc.Bacc(
        "TRN2",
        target_bir_lowering=False,
        debug=False,
        enable_asserts=False,
        num_devices=N_CORES,
    )
    # h[w, p, (t, {hi,lo}, d)] -- per-partition contiguous runs of T*2*D*2B
    h = nc.dram_tensor("h", [W, P, T * planes * D], BF16, kind="ExternalInput")
    lid = nc.dram_tensor("lid", [P, W * T], BF16, kind="ExternalInput")
    iota = nc.dram_tensor("iota", [P, P], BF16, kind="ExternalInput")
    out = nc.dram_tensor("out", [W, P, D], F32, kind="ExternalOutput")

    with tile.TileContext(nc) as tc, ExitStack() as ctx:
        const = ctx.enter_context(tc.tile_pool(name="const", bufs=1))
        hpool = ctx.enter_context(tc.tile_pool(name="hw", bufs=8))
        ohpool = ctx.enter_context(tc.tile_pool(name="oh", bufs=4))
        opool = ctx.enter_context(tc.tile_pool(name="ot", bufs=4))
        psum = ctx.enter_context(tc.tile_pool(name="acc", bufs=4, space="PSUM"))

        halves = [(0, T // 2), (T // 2, T)]

        # issue the first windows' loads before the constants so the SDMA
        # engines have bulk work immediately
        # alternate h-loads across the two HWDGE rings (SP and ACT) so
        # descriptor generation for consecutive loads overlaps
        rings = [nc.sync, nc.scalar]
        nload = 0

        def load_h(w, t0, t1):
            nonlocal nload
            ht = hpool.tile([P, (t1 - t0) * planes * D], BF16, tag="ht")
            rings[nload % 2].dma_start(
                ht[:], h[w][:, t0 * planes * D : t1 * planes * D]
            )
            nload += 1
            return ht

        hts = {}
        for w in range(2):
            for t0, t1 in halves:
                hts[(w, t0)] = load_h(w, t0, t1)

        iota_sb = const.tile([P, P], BF16)
        nc.sync.dma_start(iota_sb[:], iota[:])
        lid_sb = const.tile([P, W * T], BF16)
        nc.sync.dma_start(lid_sb[:], lid[:])

        for w in range(W):
            acc = psum.tile([P, D], F32)
            for t0, t1 in halves:
                th = t1 - t0
                if (w, t0) in hts:
                    ht = hts[(w, t0)]
                else:
                    ht = load_h(w, t0, t1)
                # one fused DVE op builds this half-window's one-hot tiles:
                # oh[p, t, v] = (iota[p, v] == lid[p, w*T + t0 + t])
                oh = ohpool.tile([P, th, P], BF16)
                nc.vector.tensor_tensor(
                    oh[:],
                    iota_sb[:].unsqueeze(1).broadcast_to((P, th, P)),
                    lid_sb[:, w * T + t0 : w * T + t1]
                    .unsqueeze(2)
                    .broadcast_to((P, th, P)),
                    mybir.AluOpType.is_equal,
                )
                for t in range(th):
                    for pl in range(planes):
                        nc.tensor.matmul(
                            acc[:],
                            oh[:, t, :],
                            ht[:, (planes * t + pl) * D : (planes * t + pl + 1) * D],
                            start=(t0 == 0 and t == 0 and pl == 0),
                            stop=(t1 == T and t == th - 1 and pl == planes - 1),
                        )
            ot = opool.tile([P, D], F32)
            nc.scalar.copy(ot[:], acc[:])
            nc.scalar.dma_start(out[w], ot[:])

    nc.compile()
    return nc


def _prepare(H: np.ndarray, X: np.ndarray, V: int, planes: int):
    """Host-side sort + greedy windowing + hi/lo split + swizzle.

    Returns (in_maps, wbase[k, w] window base segments, W, T).
    """
    N, Dd = H.shape
    assert Dd == D and N % N_CORES == 0
    nloc = N // N_CORES
    X = np.ascontiguousarray(X).astype(np.int64, copy=False)
    perm = np.argsort(X, kind="stable")
    sidx = X[perm]

    def greedy(T):
        # greedy windows per core: <=T*128 rows and <=128-segment span each
        cap = T * P
        bounds = []  # per core: row-rank boundaries [0, ..., nloc]
        for k in range(N_CORES):
            s = sidx[k * nloc : (k + 1) * nloc]
            b = [0]
            r = 0
            while r < nloc:
                r = min(r + cap, int(np.searchsorted(s, s[r] + P, side="left")))
                b.append(r)
            bounds.append(np.asarray(b, np.int64))
        return bounds, max(len(b) - 1 for b in bounds)

    best = None
    for T in T_CANDIDATES:
        bounds, W = greedy(T)
        if best is None or W * T < best[2] * best[1]:
            best = (bounds, T, W)
    bounds, T, W = best
    cap = T * P

    # per-row window index / rank / local segment id
    wbase = np.full((N_CORES, W), V, np.int64)  # pad windows point past V
    win = np.empty(N, np.int64)
    rank = np.empty(N, np.int64)
    for k in range(N_CORES):
        b = bounds[k]
        s = sidx[k * nloc : (k + 1) * nloc]
        idx = np.arange(nloc)
        wk = np.searchsorted(b, idx, side="right") - 1
        win[k * nloc : (k + 1) * nloc] = wk
        rank[k * nloc : (k + 1) * nloc] = idx - b[wk]
        wbase[k, : len(b) - 1] = s[b[:-1]]

    k_arr = np.repeat(np.arange(N_CORES), nloc)
    lid_val = sidx - wbase[k_arr, win]
    # slot layout: [core][window][partition][tile] so each partition's DRAM
    # run within a window is contiguous
    slot = (k_arr * W + win) * cap + (rank & (P - 1)) * T + (rank >> 7)

    total = N_CORES * W * cap
    src = np.zeros(total, np.int64)
    src[slot] = perm

    hi = H.astype(NP_BF16)
    Hp = np.empty((total, planes, D), NP_BF16)
    Hp[:, 0, :] = hi[src]
    if planes == 2:
        lo = (H - hi.astype(np.float32)).astype(NP_BF16)
        Hp[:, 1, :] = lo[src]
    Hp = Hp.reshape(N_CORES, W, P, T * planes * D)

    lid = np.full(total, PAD_LID, NP_BF16)
    lid[slot] = lid_val.astype(NP_BF16)
    lid = (
        lid.reshape(N_CORES, W, P, T).transpose(0, 2, 1, 3).reshape(N_CORES, P, W * T)
    )
    lid = np.ascontiguousarray(lid)

    iota = np.ascontiguousarray(
        np.broadcast_to(np.arange(P, dtype=np.float32).astype(NP_BF16), (P, P))
    )

    in_maps = [{"h": Hp[k], "lid": lid[k], "iota": iota} for k in range(N_CORES)]
    return in_maps, wbase, W, T


def kernel(H, X_node, V, trace: bool = False) -> np.ndarray:
    global LAST_RESULTS
    H = np.asarray(H, dtype=np.float32)
    X = np.asarray(X_node)
    V = int(V)

    planes = int(os.environ.get("SEGSUM_PLANES", "2"))
    in_maps, wbase, W, T = _prepare(H, X, V, planes)
    nc = _build_nc_cached(W, T, planes)
    res = run_bass_kernel_spmd(nc, in_maps, list(range(N_CORES)), trace=trace)
    LAST_RESULTS = res

    out = np.zeros((V + P, D), np.float32)
    for k in range(N_CORES):
        o = np.asarray(res.results[k]["out"])
        for w in range(W):
            b = int(wbase[k, w])
            out[b : b + P] += o[w]
    return np.ascontiguousarray(out[:V])


# revision 25
# speedup vs baseline: 1.0501x; 1.0501x over previous
"""Trainium2 (Bass/Tile) segment-sum kernel, 8-core SPMD.

Computes out[v, :] = sum over rows n with X_node[n] == v of H[n, :]
(equivalent to jax.ops.segment_sum(H, X_node, num_segments=V)).

Strategy:
  host: stable-argsort rows by segment id; split the sorted order into 8
    contiguous chunks (one per NeuronCore) so each core covers a narrow,
    contiguous segment range (~V/8 segments). Within a core, rows are
    greedily grouped into W windows, each covering <=128 consecutive
    segments and <=T*128 rows; each window is laid out as T tiles of 128
    rows, padded with dummy rows (lid=255) so all 8 cores run ONE static
    SPMD program. The DRAM layout is pre-swizzled so each window is a
    single descriptor-efficient DMA ([128 partitions] x contiguous runs).
  precision: H is split as H = hi + lo with hi = bf16(H) and
    lo = bf16(H - hi) (two bf16 planes = same DMA bytes as f32, ~2^-17
    relative representation error). The one-hot weights are exactly 0/1
    in bf16, and the PE accumulates in fp32 PSUM, so the result matches
    the f32 reference to ~1e-5 relative. bf16 matmuls run the PE at
    2.4 GHz with hidden weight loads (~59 ns per 128x128x128), vs fp32's
    ~224 ns -- this moves the kernel from PE-bound to DMA-bound.
  device, per (window w, tile t): VectorE builds the one-hot stationary
    matrix onehot[n, v] = (lid[n] == v) with one fused is_equal per
    window; TensorE accumulates PSUM[v, d] += onehot^T @ Hhi_tile
    + onehot^T @ Hlo_tile over the window's T tiles (a cross-partition
    segmented reduce); PSUM is copied to SBUF and DMA'd out per window.
  host: add the per-core [W, 128, D] window strips into the full [V, D]
    output (windows of adjacent cores may overlap; addition is exact).

Measured on the target data: ~310-330 us HW exec across 8 cores (f32 DMA
roofline ~290 us), relative error 2.5e-6 vs the f32 reference. Setting
SEGSUM_PLANES=1 ships H as a single bf16 plane instead of hi+lo
(~233 us, relative error ~1.7e-3).
"""

import os

import numpy as np
from contextlib import ExitStack

import ml_dtypes
import concourse.bass as bass
import concourse.tile as tile
from concourse import bacc, mybir
from concourse.bass_utils import run_bass_kernel_spmd

F32 = mybir.dt.float32
BF16 = mybir.dt.bfloat16
NP_BF16 = ml_dtypes.bfloat16
P = 128  # partitions / tile rows / max window width (segments)
D = 128  # feature dim
N_CORES = 8
T_CANDIDATES = (28, 29, 30, 31, 32)  # tiles (of 128 rows) per window
PAD_LID = 255.0

LAST_RESULTS = None  # test-harness hook: BassKernelResults of the last run
_NC_CACHE = {}  # (W, T, planes) -> compiled Bacc program


def _build_nc_cached(W: int, T: int, planes: int):
    key = (W, T, planes)
    if key not in _NC_CACHE:
        _NC_CACHE[key] = _build_nc(W, T, planes)
    return _NC_CACHE[key]


def _build_nc(W: int, T: int, planes: int):
    nc = bacc.Bacc(
        "TRN2",
        target_bir_lowering=False,
        debug=False,
        enable_asserts=False,
        num_devices=N_CORES,
    )
    # h[w, p, (t, {hi,lo}, d)] -- per-partition contiguous runs of T*2*D*2B
    h = nc.dram_tensor("h", [W, P, T * planes * D], BF16, kind="ExternalInput")
    lid = nc.dram_tensor("lid", [P, W * T], BF16, kind="ExternalInput")
    iota = nc.dram_tensor("iota", [P, P], BF16, kind="ExternalInput")
    out = nc.dram_tensor("out", [W, P, D], F32, kind="ExternalOutput")

    with tile.TileContext(nc) as tc, ExitStack() as ctx:
        const = ctx.enter_context(tc.tile_pool(name="const", bufs=1))
        hpool = ctx.enter_context(tc.tile_pool(name="hw", bufs=8))
        ohpool = ctx.enter_context(tc.tile_pool(name="oh", bufs=4))
        opool = ctx.enter_context(tc.tile_pool(name="ot", bufs=4))
        psum = ctx.enter_context(tc.tile_pool(name="acc", bufs=4, space="PSUM"))

        halves = [(0, T // 2), (T // 2, T)]

        # issue the first windows' loads before the constants so the SDMA
        # engines have bulk work immediately
        def load_h(w, t0, t1):
            ht = hpool.tile([P, (t1 - t0) * planes * D], BF16, tag="ht")
            nc.sync.dma_start(ht[:], h[w][:, t0 * planes * D : t1 * planes * D])
            return ht

        hts = {}
        for w in range(2):
            for t0, t1 in halves:
                hts[(w, t0)] = load_h(w, t0, t1)

        iota_sb = const.tile([P, P], BF16)
        nc.sync.dma_start(iota_sb[:], iota[:])
        lid_sb = const.tile([P, W * T], BF16)
        nc.sync.dma_start(lid_sb[:], lid[:])

        for w in range(W):
            # one wide matmul per tile streams all planes; the psum holds
            # per-plane partial sums side by side, added at window end
            acc = psum.tile([P, planes * D], F32)
            for t0, t1 in halves:
                th = t1 - t0
                if (w, t0) in hts:
                    ht = hts[(w, t0)]
                else:
                    ht = load_h(w, t0, t1)
                # one fused DVE op builds this half-window's one-hot tiles:
                # oh[p, t, v] = (iota[p, v] == lid[p, w*T + t0 + t])
                oh = ohpool.tile([P, th, P], BF16)
                nc.vector.tensor_tensor(
                    oh[:],
                    iota_sb[:].unsqueeze(1).broadcast_to((P, th, P)),
                    lid_sb[:, w * T + t0 : w * T + t1]
                    .unsqueeze(2)
                    .broadcast_to((P, th, P)),
                    mybir.AluOpType.is_equal,
                )
                for t in range(th):
                    nc.tensor.matmul(
                        acc[:],
                        oh[:, t, :],
                        ht[:, planes * t * D : planes * (t + 1) * D],
                        start=(t0 == 0 and t == 0),
                        stop=(t1 == T and t == th - 1),
                    )
            ot = opool.tile([P, D], F32)
            nc.scalar.copy(ot[:], acc[:, :D])
            if planes == 2:
                # DVE allows only one PSUM operand per op
                nc.vector.tensor_tensor(
                    ot[:], ot[:], acc[:, D:], mybir.AluOpType.add
                )
            nc.scalar.dma_start(out[w], ot[:])

    nc.compile()
    return nc


def _prepare(H: np.ndarray, X: np.ndarray, V: int, planes: int):
    """Host-side sort + greedy windowing + hi/lo split + swizzle.

    Returns (in_maps, wbase[k, w] window base segments, W, T).
    """
    N, Dd = H.shape
    assert Dd == D and N % N_CORES == 0
    nloc = N // N_CORES
    X = np.ascontiguousarray(X).astype(np.int64, copy=False)
    perm = np.argsort(X, kind="stable")
    sidx = X[perm]

    def greedy(T):
        # greedy windows per core: <=T*128 rows and <=128-segment span each
        cap = T * P
        bounds = []  # per core: row-rank boundaries [0, ..., nloc]
        for k in range(N_CORES):
            s = sidx[k * nloc : (k + 1) * nloc]
            b = [0]
            r = 0
            while r < nloc:
                r = min(r + cap, int(np.searchsorted(s, s[r] + P, side="left")))
                b.append(r)
            bounds.append(np.asarray(b, np.int64))
        return bounds, max(len(b) - 1 for b in bounds)

    best = None
    for T in T_CANDIDATES:
        bounds, W = greedy(T)
        if best is None or W * T < best[2] * best[1]:
            best = (bounds, T, W)
    bounds, T, W = best
    cap = T * P

    # per-row window index / rank / local segment id
    wbase = np.full((N_CORES, W), V, np.int64)  # pad windows point past V
    win = np.empty(N, np.int64)
    rank = np.empty(N, np.int64)
    for k in range(N_CORES):
        b = bounds[k]
        s = sidx[k * nloc : (k + 1) * nloc]
        idx = np.arange(nloc)
        wk = np.searchsorted(b, idx, side="right") - 1
        win[k * nloc : (k + 1) * nloc] = wk
        rank[k * nloc : (k + 1) * nloc] = idx - b[wk]
        wbase[k, : len(b) - 1] = s[b[:-1]]

    k_arr = np.repeat(np.arange(N_CORES), nloc)
    lid_val = sidx - wbase[k_arr, win]
    # slot layout: [core][window][partition][tile] so each partition's DRAM
    # run within a window is contiguous
    slot = (k_arr * W + win) * cap + (rank & (P - 1)) * T + (rank >> 7)

    total = N_CORES * W * cap
    src = np.zeros(total, np.int64)
    src[slot] = perm

    hi = H.astype(NP_BF16)
    Hp = np.empty((total, planes, D), NP_BF16)
    Hp[:, 0, :] = hi[src]
    if planes == 2:
        lo = (H - hi.astype(np.float32)).astype(NP_BF16)
        Hp[:, 1, :] = lo[src]
    Hp = Hp.reshape(N_CORES, W, P, T * planes * D)

    lid = np.full(total, PAD_LID, NP_BF16)
    lid[slot] = lid_val.astype(NP_BF16)
    lid = (
        lid.reshape(N_CORES, W, P, T).transpose(0, 2, 1, 3).reshape(N_CORES, P, W * T)
    )
    lid = np.ascontiguousarray(lid)

    iota = np.ascontiguousarray(
        np.broadcast_to(np.arange(P, dtype=np.float32).astype(NP_BF16), (P, P))
    )

    in_maps = [{"h": Hp[k], "lid": lid[k], "iota": iota} for k in range(N_CORES)]
    return in_maps, wbase, W, T


def kernel(H, X_node, V, trace: bool = False) -> np.ndarray:
    global LAST_RESULTS
    H = np.asarray(H, dtype=np.float32)
    X = np.asarray(X_node)
    V = int(V)

    planes = int(os.environ.get("SEGSUM_PLANES", "2"))
    in_maps, wbase, W, T = _prepare(H, X, V, planes)
    nc = _build_nc_cached(W, T, planes)
    res = run_bass_kernel_spmd(
        nc,
        in_maps,
        list(range(N_CORES)),
        trace=trace,
        trace_cores=list(range(N_CORES)) if trace else None,
    )
    LAST_RESULTS = res

    out = np.zeros((V + P, D), np.float32)
    for k in range(N_CORES):
        o = np.asarray(res.results[k]["out"])
        for w in range(W):
            b = int(wbase[k, w])
            out[b : b + P] += o[w]
    return np.ascontiguousarray(out[:V])


# revision 26
# speedup vs baseline: 1.2101x; 1.1524x over previous
"""Trainium2 (Bass/Tile) segment-sum kernel, 8-core SPMD.

Computes out[v, :] = sum over rows n with X_node[n] == v of H[n, :]
(equivalent to jax.ops.segment_sum(H, X_node, num_segments=V)).

Strategy:
  host: stable-argsort rows by segment id; split the sorted order into 8
    contiguous chunks (one per NeuronCore) so each core covers a narrow,
    contiguous segment range (~V/8 segments). Within a core, rows are
    greedily grouped into W windows, each covering <=128 consecutive
    segments and <=T*128 rows; each window is laid out as T tiles of 128
    rows, padded with dummy rows (lid=255) so all 8 cores run ONE static
    SPMD program. The DRAM layout is pre-swizzled so each window is a
    single descriptor-efficient DMA ([128 partitions] x contiguous runs).
  precision: H is split as H = hi + lo with hi = bf16(H) and
    lo = bf16(H - hi) (two bf16 planes = same DMA bytes as f32, ~2^-17
    relative representation error). The one-hot weights are exactly 0/1
    in bf16, and the PE accumulates in fp32 PSUM, so the result matches
    the f32 reference to ~1e-5 relative. bf16 matmuls run the PE at
    2.4 GHz with hidden weight loads (~59 ns per 128x128x128), vs fp32's
    ~224 ns -- this moves the kernel from PE-bound to DMA-bound.
  device, per (window w, tile t): VectorE builds the one-hot stationary
    matrix onehot[n, v] = (lid[n] == v) with one fused is_equal per
    window; TensorE accumulates PSUM[v, d] += onehot^T @ Hhi_tile
    + onehot^T @ Hlo_tile over the window's T tiles (a cross-partition
    segmented reduce); PSUM is copied to SBUF and DMA'd out per window.
  host: add the per-core [W, 128, D] window strips into the full [V, D]
    output (windows of adjacent cores may overlap; addition is exact).

Measured on the target data: ~310-330 us HW exec across 8 cores (f32 DMA
roofline ~290 us), relative error 2.5e-6 vs the f32 reference. Setting
SEGSUM_PLANES=1 ships H as a single bf16 plane instead of hi+lo
(~233 us, relative error ~1.7e-3).
"""

import os

import numpy as np
from contextlib import ExitStack

import ml_dtypes
import concourse.bass as bass
import concourse.tile as tile
from concourse import bacc, mybir
from concourse.bass_utils import run_bass_kernel_spmd

F32 = mybir.dt.float32
BF16 = mybir.dt.bfloat16
NP_BF16 = ml_dtypes.bfloat16
P = 128  # partitions / tile rows / max window width (segments)
D = 128  # feature dim
N_CORES = 8
T_CANDIDATES = (28, 29, 30, 31, 32)  # tiles (of 128 rows) per window
PAD_LID = 255.0

LAST_RESULTS = None  # test-harness hook: BassKernelResults of the last run
_NC_CACHE = {}  # (W, T, planes) -> compiled Bacc program


def _build_nc_cached(W: int, T: int, planes: int):
    key = (W, T, planes)
    if key not in _NC_CACHE:
        _NC_CACHE[key] = _build_nc(W, T, planes)
    return _NC_CACHE[key]


def _build_nc(W: int, T: int, planes: int):
    nc = bacc.Bacc(
        "TRN2",
        target_bir_lowering=False,
        debug=False,
        enable_asserts=False,
        num_devices=N_CORES,
    )
    # h[w, p, (t, {hi,lo}, d)] -- per-partition contiguous runs of T*2*D*2B
    h = nc.dram_tensor("h", [W, P, T * planes * D], BF16, kind="ExternalInput")
    lid = nc.dram_tensor("lid", [P, W * T], BF16, kind="ExternalInput")
    iota = nc.dram_tensor("iota", [P, P], BF16, kind="ExternalInput")
    out = nc.dram_tensor("out", [W, P, D], F32, kind="ExternalOutput")

    with tile.TileContext(nc) as tc, ExitStack() as ctx:
        const = ctx.enter_context(tc.tile_pool(name="const", bufs=1))
        hpool = ctx.enter_context(tc.tile_pool(name="hw", bufs=8))
        ohpool = ctx.enter_context(tc.tile_pool(name="oh", bufs=4))
        opool = ctx.enter_context(tc.tile_pool(name="ot", bufs=4))
        psum = ctx.enter_context(tc.tile_pool(name="acc", bufs=4, space="PSUM"))

        halves = [(0, T // 2), (T // 2, T)]

        # issue the first windows' loads before the constants so the SDMA
        # engines have bulk work immediately
        def load_h(w, t0, t1):
            ht = hpool.tile([P, (t1 - t0) * planes * D], BF16, tag="ht")
            nc.sync.dma_start(ht[:], h[w][:, t0 * planes * D : t1 * planes * D])
            return ht

        hts = {}
        for w in range(2):
            for t0, t1 in halves:
                hts[(w, t0)] = load_h(w, t0, t1)

        iota_sb = const.tile([P, P], BF16)
        nc.sync.dma_start(iota_sb[:], iota[:])
        lid_sb = const.tile([P, W * T], BF16)
        nc.sync.dma_start(lid_sb[:], lid[:])

        for w in range(W):
            # one wide matmul per tile streams all planes; the psum holds
            # per-plane partial sums side by side, added at window end
            acc = psum.tile([P, planes * D], F32)
            for t0, t1 in halves:
                th = t1 - t0
                if (w, t0) in hts:
                    ht = hts[(w, t0)]
                else:
                    ht = load_h(w, t0, t1)
                # one fused DVE op builds this half-window's one-hot tiles:
                # oh[p, t, v] = (iota[p, v] == lid[p, w*T + t0 + t])
                oh = ohpool.tile([P, th, P], BF16)
                nc.vector.tensor_tensor(
                    oh[:],
                    iota_sb[:].unsqueeze(1).broadcast_to((P, th, P)),
                    lid_sb[:, w * T + t0 : w * T + t1]
                    .unsqueeze(2)
                    .broadcast_to((P, th, P)),
                    mybir.AluOpType.is_equal,
                )
                for t in range(th):
                    nc.tensor.matmul(
                        acc[:],
                        oh[:, t, :],
                        ht[:, planes * t * D : planes * (t + 1) * D],
                        start=(t0 == 0 and t == 0),
                        stop=(t1 == T and t == th - 1),
                    )
            ot = opool.tile([P, D], F32)
            nc.scalar.copy(ot[:], acc[:, :D])
            if planes == 2:
                # DVE allows only one PSUM operand per op
                nc.vector.tensor_tensor(
                    ot[:], ot[:], acc[:, D:], mybir.AluOpType.add
                )
            nc.scalar.dma_start(out[w], ot[:])

    nc.compile()
    return nc


def _prepare(H: np.ndarray, X: np.ndarray, V: int, planes: int):
    """Host-side sort + greedy windowing + hi/lo split + swizzle.

    Returns (in_maps, wbase[k, w] window base segments, W, T).
    """
    N, Dd = H.shape
    assert Dd == D and N % N_CORES == 0
    nloc = N // N_CORES
    X = np.ascontiguousarray(X).astype(np.int64, copy=False)
    perm = np.argsort(X, kind="stable")
    sidx = X[perm]

    def greedy(T):
        # greedy windows per core: <=T*128 rows and <=128-segment span each
        cap = T * P
        bounds = []  # per core: row-rank boundaries [0, ..., nloc]
        for k in range(N_CORES):
            s = sidx[k * nloc : (k + 1) * nloc]
            b = [0]
            r = 0
            while r < nloc:
                r = min(r + cap, int(np.searchsorted(s, s[r] + P, side="left")))
                b.append(r)
            bounds.append(np.asarray(b, np.int64))
        return bounds, max(len(b) - 1 for b in bounds)

    best = None
    for T in T_CANDIDATES:
        bounds, W = greedy(T)
        if best is None or W * T < best[2] * best[1]:
            best = (bounds, T, W)
    bounds, T, W = best
    cap = T * P

    # per-row window index / rank / local segment id
    wbase = np.full((N_CORES, W), V, np.int64)  # pad windows point past V
    win = np.empty(N, np.int64)
    rank = np.empty(N, np.int64)
    for k in range(N_CORES):
        b = bounds[k]
        s = sidx[k * nloc : (k + 1) * nloc]
        idx = np.arange(nloc)
        wk = np.searchsorted(b, idx, side="right") - 1
        win[k * nloc : (k + 1) * nloc] = wk
        rank[k * nloc : (k + 1) * nloc] = idx - b[wk]
        wbase[k, : len(b) - 1] = s[b[:-1]]

    k_arr = np.repeat(np.arange(N_CORES), nloc)
    lid_val = sidx - wbase[k_arr, win]
    # slot layout: [core][window][partition][tile] so each partition's DRAM
    # run within a window is contiguous
    slot = (k_arr * W + win) * cap + (rank & (P - 1)) * T + (rank >> 7)

    total = N_CORES * W * cap
    src = np.zeros(total, np.int64)
    src[slot] = perm

    hi = H.astype(NP_BF16)
    Hp = np.empty((total, planes, D), NP_BF16)
    Hp[:, 0, :] = hi[src]
    if planes == 2:
        lo = (H - hi.astype(np.float32)).astype(NP_BF16)
        Hp[:, 1, :] = lo[src]
    Hp = Hp.reshape(N_CORES, W, P, T * planes * D)

    lid = np.full(total, PAD_LID, NP_BF16)
    lid[slot] = lid_val.astype(NP_BF16)
    lid = (
        lid.reshape(N_CORES, W, P, T).transpose(0, 2, 1, 3).reshape(N_CORES, P, W * T)
    )
    lid = np.ascontiguousarray(lid)

    iota = np.ascontiguousarray(
        np.broadcast_to(np.arange(P, dtype=np.float32).astype(NP_BF16), (P, P))
    )

    in_maps = [{"h": Hp[k], "lid": lid[k], "iota": iota} for k in range(N_CORES)]
    return in_maps, wbase, W, T


def kernel(H, X_node, V, trace: bool = False) -> np.ndarray:
    global LAST_RESULTS
    H = np.asarray(H, dtype=np.float32)
    X = np.asarray(X_node)
    V = int(V)

    planes = int(os.environ.get("SEGSUM_PLANES", "2"))
    in_maps, wbase, W, T = _prepare(H, X, V, planes)
    nc = _build_nc_cached(W, T, planes)
    res = run_bass_kernel_spmd(nc, in_maps, list(range(N_CORES)), trace=trace)
    LAST_RESULTS = res

    out = np.zeros((V + P, D), np.float32)
    for k in range(N_CORES):
        o = np.asarray(res.results[k]["out"])
        for w in range(W):
            b = int(wbase[k, w])
            out[b : b + P] += o[w]
    return np.ascontiguousarray(out[:V])
